# revision 1
# baseline (speedup 1.0000x reference)
"""CIN (Compressed Interaction Network) kernel for Trainium2, 8 NeuronCores.

Problem: x (2048, 39, 16) f32; 3 CIN layers with W_i (200, 39, prev):
    z[b,o,d] = sum_{f,g} W[o,f,g] * x0[b,f,d] * h[b,g,d] + bias[o]
    h' = relu(z);  output = sum_d concat([h1,h2,h3], ch) -> (2048, 600)

Strategy (data-parallel over batch, 8 cores, 256 batch rows each):
  Per core, columns n = (b_local, d), N = 256*16 = 4096, in 8 n-tiles of 512.
  z[:, n] = sum_f (W[:, f, :] @ (h ⊙ bcast(x0[f, :]))) — the Khatri-Rao
  factor V_f = h ⊙ x0[f] is built on the Vector engine (fp16 tensor_tensor,
  widened 4 f's per instruction) against a partition-replicated x0 tile
  (built per n-tile via broadcast DMA straight from DRAM), and consumed by
  per-f fp16 matmuls accumulated in PSUM over (f, g-chunks).  Layer 0 packs
  two 39-row f-blocks per 128-row chunk (rows 0:64 / 64:128) against a
  zero-padded x0 tile.  n-tiles are emitted pairwise-interleaved so the PE
  always has an independent tile's matmuls to run across layer boundaries.
  Bias+relu fused on the Scalar engine; d-sums on the Vector engine; weight
  re-layout and the output transpose happen on host.
"""
import numpy as np

import concourse.bacc as bacc
import concourse.mybir as mybir
import concourse.tile as tile
from concourse.bass_utils import run_bass_kernel_spmd

B, F0, D = 2048, 39, 16
C = 200                      # cross size per layer
NCORES = 8
BC = B // NCORES             # 256 batch rows per core
N = BC * D                   # 4096 columns per core
NT = 512                     # n-tile width
T = N // NT                  # 8 n-tiles
BT = NT // D                 # 32 batch rows per n-tile
OA, OB = 128, C - 128        # output-channel chunks (128 + 72)
NC0 = (F0 + 1) // 2          # legacy packed-L0 chunk count (unused)
KF0 = (F0 * F0 + 127) // 128  # 12 flat L0 K-chunks (v0 host-built, zero-padded)
WJ = 4                       # f's per widened tensor_tensor
F16 = mybir.dt.float16
F32 = mybir.dt.float32
MUL = None  # set lazily


def _build_nc(reps=1):
    nc = bacc.Bacc(None, target_bir_lowering=False)
    mult = mybir.AluOpType.mult
    relu = mybir.ActivationFunctionType.Relu

    x0_d = nc.dram_tensor("x0", [F0, N], F16, kind="ExternalInput")
    v0_d = nc.dram_tensor("v0", [KF0 * 128, N], F16, kind="ExternalInput")
    w0_d = nc.dram_tensor("w0", [128, KF0 * C], F16, kind="ExternalInput")
    w1a_d = nc.dram_tensor("w1a", [OA, F0 * C], F16, kind="ExternalInput")
    w1b_d = nc.dram_tensor("w1b", [OB, F0 * C], F16, kind="ExternalInput")
    w2a_d = nc.dram_tensor("w2a", [OA, F0 * C], F16, kind="ExternalInput")
    w2b_d = nc.dram_tensor("w2b", [OB, F0 * C], F16, kind="ExternalInput")
    b_d = nc.dram_tensor("b", [3 * C, 1], F32, kind="ExternalInput")
    out_d = nc.dram_tensor("out3", [3, C, BC], F32, kind="ExternalOutput")

    with tile.TileContext(nc) as tc:
        with (
            tc.tile_pool(name="wp", bufs=1) as wp,
            tc.tile_pool(name="bc", bufs=2) as bcp,
            tc.tile_pool(name="hp", bufs=2) as hp,
            tc.tile_pool(name="va", bufs=3) as vap,
            tc.tile_pool(name="vb", bufs=2) as vbp,
            tc.tile_pool(name="ps", bufs=2, space="PSUM") as ps,
        ):
            # --- static state -------------------------------------------------
            w0 = wp.tile([128, KF0 * C], F16)
            nc.sync.dma_start(out=w0[:], in_=w0_d[:])
            biases = []
            for l in range(3):
                ba = wp.tile([OA, 1], F32, tag=f"b{l}a")
                bb = wp.tile([OB, 1], F32, tag=f"b{l}b")
                nc.sync.dma_start(out=ba[:], in_=b_d[l * C:l * C + OA, :])
                nc.sync.dma_start(out=bb[:], in_=b_d[l * C + OA:(l + 1) * C, :])
                biases.append((ba, bb))
            outs = []
            for l in range(3):
                oa = wp.tile([OA, BC], F32, tag=f"o{l}a")
                ob = wp.tile([OB, BC], F32, tag=f"o{l}b")
                outs.append((oa, ob))

            def emit_xb(t, fchunk=10):
                # split into f-chunks so consumers start before the whole
                # 5MB replication lands
                xb = bcp.tile([128, F0 * NT], F16, tag="xb")
                for f0 in range(0, F0, fchunk):
                    f1 = min(f0 + fchunk, F0)
                    src = (x0_d[f0:f1, t * NT:(t + 1) * NT]
                           .unsqueeze(0).broadcast_to((128, f1 - f0, NT)))
                    nc.sync.dma_start(
                        out=xb[:, f0 * NT:f1 * NT]
                        .rearrange("p (f n) -> p f n", n=NT), in_=src)
                return xb

            def emit_v0(t):
                # host-computed Khatri-Rao x0⊗x0, flat (f*39+g) rows,
                # zero-padded to KF0*128; one strided DMA per n-tile
                v0t = bcp.tile([128, KF0 * NT], F16, tag="v0t")
                src = (v0_d[:].rearrange("(c p) n -> p c n", p=128)
                       [:, :, t * NT:(t + 1) * NT])
                for c0 in range(0, KF0, 4):
                    c1 = min(c0 + 4, KF0)
                    nc.sync.dma_start(
                        out=v0t[:, c0 * NT:c1 * NT]
                        .rearrange("p (c n) -> p c n", n=NT),
                        in_=src[:, c0:c1, :])
                return v0t

            def emit_l0(t, v0t):
                pa = ps.tile([OA, NT], F32, tag="pa")
                pb = ps.tile([OB, NT], F32, tag="pb")
                for c in range(KF0):
                    first, last = c == 0, c == KF0 - 1
                    os_a = slice(c * C, c * C + OA)
                    os_b = slice(c * C + OA, (c + 1) * C)
                    rhs = v0t[:, c * NT:(c + 1) * NT]
                    nc.tensor.matmul(pa[:], w0[:, os_a], rhs,
                                     start=first, stop=last)
                    nc.tensor.matmul(pb[:], w0[:, os_b], rhs,
                                     start=first, stop=last)
                return pa, pb

            def emit_l12(xb, wa, wb, ha, hb):
                pa = ps.tile([OA, NT], F32, tag="pa")
                pb = ps.tile([OB, NT], F32, tag="pb")
                for j in range(0, F0, WJ):
                    w = min(WJ, F0 - j)
                    fs = slice(j * NT, (j + w) * NT)
                    va = vap.tile([OA, WJ * NT], F16, tag="v4a")
                    vb = vbp.tile([OB, WJ * NT], F16, tag="v4b")
                    nc.vector.tensor_tensor(
                        out=va[:, 0:w * NT].rearrange("p (f n) -> p f n", n=NT),
                        in0=ha[:].unsqueeze(1).broadcast_to((OA, w, NT)),
                        in1=xb[0:OA, fs].rearrange("p (f n) -> p f n", n=NT),
                        op=mult)
                    nc.vector.tensor_tensor(
                        out=vb[:, 0:w * NT].rearrange("p (f n) -> p f n", n=NT),
                        in0=hb[:].unsqueeze(1).broadcast_to((OB, w, NT)),
                        in1=xb[0:OB, fs].rearrange("p (f n) -> p f n", n=NT),
                        op=mult)
                    for i in range(w):
                        f = j + i
                        first, last = f == 0, f == F0 - 1
                        os_a = slice(f * C, f * C + OA)
                        os_b = slice(f * C + OA, (f + 1) * C)
                        ra = va[:, i * NT:(i + 1) * NT]
                        rb = vb[:, i * NT:(i + 1) * NT]
                        nc.tensor.matmul(pa[:], wa[:, os_a], ra,
                                         start=first, stop=False)
                        nc.tensor.matmul(pa[:], wb[:, os_a], rb,
                                         start=False, stop=last)
                        nc.tensor.matmul(pb[:], wa[:, os_b], ra,
                                         start=first, stop=False)
                        nc.tensor.matmul(pb[:], wb[:, os_b], rb,
                                         start=False, stop=last)
                return pa, pb

            def emit_epi(t, l, pa, pb):
                ba, bb = biases[l]
                ha = hp.tile([OA, NT], F16, tag="ha")
                hb = hp.tile([OB, NT], F16, tag="hb")
                nc.scalar.activation(ha[:], pa[:], relu, bias=ba[:])
                nc.scalar.activation(hb[:], pb[:], relu, bias=bb[:])
                oa, ob = outs[l]
                bs = slice(t * BT, (t + 1) * BT)
                nc.vector.tensor_reduce(
                    out=oa[:, bs], in_=ha[:].rearrange("p (b d) -> p b d", d=D),
                    axis=mybir.AxisListType.X, op=mybir.AluOpType.add)
                nc.vector.tensor_reduce(
                    out=ob[:, bs], in_=hb[:].rearrange("p (b d) -> p b d", d=D),
                    axis=mybir.AxisListType.X, op=mybir.AluOpType.add)
                return ha, hb

            # --- pipeline: pairwise-interleaved n-tiles ----------------------
            v00 = emit_v0(0)
            v01 = emit_v0(1)
            xb0 = emit_xb(0)
            xb1 = emit_xb(1)
            # big weights load after the first xb/v0 batches so the PE can start
            w1a = wp.tile([OA, F0 * C], F16)
            nc.sync.dma_start(out=w1a[:], in_=w1a_d[:])
            w1b = wp.tile([OB, F0 * C], F16)
            nc.sync.dma_start(out=w1b[:], in_=w1b_d[:])
            w2a = wp.tile([OA, F0 * C], F16)
            nc.sync.dma_start(out=w2a[:], in_=w2a_d[:])
            w2b = wp.tile([OB, F0 * C], F16)
            nc.sync.dma_start(out=w2b[:], in_=w2b_d[:])

            import contextlib
            loop_cm = tc.For_i(0, reps, 1) if reps > 1 else contextlib.nullcontext()
            with loop_cm:
              if reps > 1:
                v00 = emit_v0(0)
                v01 = emit_v0(1)
                xb0 = emit_xb(0)
                xb1 = emit_xb(1)
              for tp in range(0, T, 2):
                t0, t1 = tp, tp + 1
                p0 = emit_l0(t0, v00)
                h0 = emit_epi(t0, 0, *p0)
                p1 = emit_l0(t1, v01)
                h1 = emit_epi(t1, 0, *p1)
                p0 = emit_l12(xb0, w1a, w1b, *h0)
                h0 = emit_epi(t0, 1, *p0)
                p1 = emit_l12(xb1, w1a, w1b, *h1)
                h1 = emit_epi(t1, 1, *p1)
                p0 = emit_l12(xb0, w2a, w2b, *h0)
                emit_epi(t0, 2, *p0)
                p1 = emit_l12(xb1, w2a, w2b, *h1)
                emit_epi(t1, 2, *p1)
                if tp + 2 < T:
                    xb0 = emit_xb(tp + 2)
                    xb1 = emit_xb(tp + 3)
                    v00 = emit_v0(tp + 2)
                    v01 = emit_v0(tp + 3)

            for l in range(3):
                oa, ob = outs[l]
                nc.sync.dma_start(out=out_d[l, 0:OA, :], in_=oa[:])
                nc.sync.dma_start(out=out_d[l, OA:C, :], in_=ob[:])

    nc.compile()
    return nc


_NC_CACHE = None


def _get_nc():
    global _NC_CACHE
    if _NC_CACHE is None:
        _NC_CACHE = _build_nc()
    return _NC_CACHE


def _prep_weights(W0, W1, W2, b0, b1, b2):
    # lhsT layout: w[g, f*C + o] = W[o, f, g]
    def lay(W):
        return np.ascontiguousarray(
            W.transpose(2, 1, 0).reshape(W.shape[2], F0 * C)).astype(np.float16)

    # flat L0 lhsT: w0[p, c*C+o] = W0[o, f, g] at flat row k = 128c+p = f*39+g
    W0 = np.asarray(W0)
    w0f = np.zeros((KF0 * 128, C), np.float32)
    w0f[0:F0 * F0] = W0.reshape(C, F0 * F0).T    # [k, o], k = f*39+g
    w0 = np.ascontiguousarray(
        w0f.reshape(KF0, 128, C).transpose(1, 0, 2).reshape(128, KF0 * C)
    ).astype(np.float16)
    w1 = lay(np.asarray(W1))          # [200, 7800]
    w2 = lay(np.asarray(W2))
    b = np.concatenate([np.asarray(b0), np.asarray(b1), np.asarray(b2)])
    return {
        "w0": w0,
        "w1a": np.ascontiguousarray(w1[:OA]), "w1b": np.ascontiguousarray(w1[OA:]),
        "w2a": np.ascontiguousarray(w2[:OA]), "w2b": np.ascontiguousarray(w2[OA:]),
        "b": b.astype(np.float32).reshape(3 * C, 1),
    }


def kernel(x, W0, b0, W1, b1, W2, b2):
    x = np.asarray(x)
    assert x.shape == (B, F0, D), x.shape
    nc = _get_nc()
    shared = _prep_weights(W0, W1, W2, b0, b1, b2)

    in_maps = []
    for c in range(NCORES):
        xc = x[c * BC:(c + 1) * BC]                      # [256, 39, 16]
        x0c = np.ascontiguousarray(
            xc.transpose(1, 0, 2).reshape(F0, N)).astype(np.float16)
        x0f32 = x0c.astype(np.float32)
        v0 = np.zeros((KF0 * 128, N), np.float16)
        v0[0:F0 * F0] = (x0f32[:, None, :] * x0f32[None, :, :]
                         ).reshape(F0 * F0, N).astype(np.float16)
        in_maps.append({"x0": x0c, "v0": v0, **shared})

    res = run_bass_kernel_spmd(nc, in_maps, list(range(NCORES)))

    out = np.empty((B, 3 * C), dtype=np.float32)
    for c in range(NCORES):
        o3 = res.results[c]["out3"]                      # [3, 200, 256]
        out[c * BC:(c + 1) * BC] = o3.transpose(2, 0, 1).reshape(BC, 3 * C)
    return out

